# revision 33
# baseline (speedup 1.0000x reference)
import sys

sys.path.insert(0, "/opt/trn_rl_repo")

import numpy as np
import ml_dtypes

import concourse.bass as bass
import concourse.bacc as bacc
import concourse.tile as tile
from concourse.bass_utils import run_bass_kernel_spmd
from concourse import mybir

B, L, D, H = 2, 2048, 1024, 16
DH = 64          # dim per head
HPC = 4          # heads per core
CPC = HPC * DH   # feature cols per core = 256
NCORES = 8

MM_DT = "bfloat16"
NP_MM = ml_dtypes.bfloat16 if MM_DT == "bfloat16" else np.float32

_CACHE = {}


def build_nc(mm_dt: str):
    nc = bacc.Bacc()
    mm_dt = mybir.dt(mm_dt)
    fp32 = mybir.dt.float32

    # host pre-swizzles every input so each DMA is CONTIGUOUS per partition
    # (sync-engine DIRECT2D descriptor-gen cost scales with line count; the
    # naive "(dc p) c -> p dc c" patterns cost 3-4us of sync time EACH and
    # stall every semaphore relay queued behind them)
    xq = nc.declare_dram_parameter("xq", (128, 8 * L), mm_dt, isOutput=False)
    xk = nc.declare_dram_parameter("xk", (128, 8 * L), mm_dt, isOutput=False)
    xv = nc.declare_dram_parameter("xv", (128, 4 * 8 * 512), mm_dt, isOutput=False)
    wq = nc.declare_dram_parameter("wq", (128, 8 * CPC), mm_dt, isOutput=False)
    wk = nc.declare_dram_parameter("wk", (128, 8 * CPC), mm_dt, isOutput=False)
    wv = nc.declare_dram_parameter("wv", (128, 8 * CPC), mm_dt, isOutput=False)
    wo = nc.declare_dram_parameter("wo", (CPC, D), mm_dt, isOutput=False)
    # packed biases: cols 0:2 = bq (cc0,cc1), cols 2:4 = bk -- single fat
    # descriptor instead of 256 4-byte ones
    bqk = nc.declare_dram_parameter("bqk", (128, 4), fp32, isOutput=False)
    y = nc.declare_dram_parameter("y", (L, D), mm_dt, isOutput=True)      # partial out (bf16)

    from contextlib import ExitStack

    with ExitStack() as es:
        tc = es.enter_context(tile.TileContext(nc))
        # NOTE: bufs are per named tag
        xt_pool = es.enter_context(tc.tile_pool(name="xt", bufs=1))     # xq/xk [128,8,2048]
        xv_pool = es.enter_context(tc.tile_pool(name="xv", bufs=1))     # 4 tags [128,8,512]
        w_pool = es.enter_context(tc.tile_pool(name="w", bufs=1))       # wk/wq halves + wv
        wo_pool = es.enter_context(tc.tile_pool(name="wo", bufs=1))     # 2 tags [128,1024]
        bias_pool = es.enter_context(tc.tile_pool(name="bias", bufs=1))
        # NOTE: matmul weight-reads (ldweights) get a conservative sync dep
        # on the LAST write to the same pool at emission time, not the
        # precise slice. Separate pools per consumer group so e.g. the
        # C(2) fillers never wait on g4=3's divisions.
        qt_pools = [es.enter_context(tc.tile_pool(name=f"qt{i}", bufs=1))
                    for i in range(2)]
        kt_pools = [es.enter_context(tc.tile_pool(name=f"kt{i}", bufs=1))
                    for i in range(2)]
        vn_pools = [es.enter_context(tc.tile_pool(name=f"vn{g}", bufs=1))
                    for g in range(4)]
        pt_pool = es.enter_context(tc.tile_pool(name="pt", bufs=3))     # [128,1024] bf16
        zr_pool = es.enter_context(tc.tile_pool(name="zr", bufs=2))     # [1,512] z rows
        zb_pool = es.enter_context(tc.tile_pool(name="zb", bufs=2))     # [64,512] bcast
        zc_pool = es.enter_context(tc.tile_pool(name="zc", bufs=2))     # [64,512] recip
        ot_pools = [es.enter_context(tc.tile_pool(name=f"otp{g}", bufs=1))
                    for g in range(4)]                                  # 2 tags [128,512] each
        y_pool = es.enter_context(tc.tile_pool(name="ysb", bufs=8))     # yh [128,512] / yt [128,1024]
        psA = es.enter_context(tc.tile_pool(name="psA", bufs=2, space="PSUM"))   # 2 banks
        psS = es.enter_context(tc.tile_pool(name="psS", bufs=2, space="PSUM"))   # [128,1024] x2 = 4 banks
        psOT = es.enter_context(tc.tile_pool(name="psOT", bufs=1, space="PSUM"))  # 2 tags [65,512] = 2 banks

        # ---- SBUF staging tiles ----------------------------------------
        # w tiles split into dc-halves: qk-unit weight reads are
        # conservative whole-tile deps, so a single [128,8,CPC] tile makes
        # the FIRST matmul wait for the LAST wk DMA (~2.4us of dead start)
        wk_sb = [w_pool.tile([128, 4, CPC], mm_dt, name=f"wk{h}") for h in range(2)]
        wq_sb = [w_pool.tile([128, 4, CPC], mm_dt, name=f"wq{h}") for h in range(2)]
        wv_sb = w_pool.tile([128, 8, CPC], mm_dt, name="wv")   # moving reads: precise
        xk_sb = xt_pool.tile([128, 8, L], mm_dt, name="xk")
        xq_sb = xt_pool.tile([128, 8, L], mm_dt, name="xq")
        # xv is the STATIONARY operand of v-units -> per-lg tiles so v(0..3)
        # only wait on the first xv chunk
        xv_sb = [xv_pool.tile([128, 8, 512], mm_dt, name=f"xv{g}") for g in range(4)]

        wk_r = wk.rearrange("p (dc c) -> p dc c", dc=8)
        wq_r = wq.rearrange("p (dc c) -> p dc c", dc=8)
        wv_r = wv.rearrange("p (dc c) -> p dc c", dc=8)
        xk_r = xk.rearrange("p (dc c) -> p dc c", dc=8)
        xq_r = xq.rearrange("p (dc c) -> p dc c", dc=8)
        xv_r = xv.rearrange("p (g dc c) -> p g dc c", g=4, dc=8)

        # TWO DMA issue rings: the K-side on sync (SP), the Q-side on ACT
        # (idle until the first exp at ~28us). Descriptor gen (DIRECT2D) is
        # ~0.6us per dma_start and serializes per engine -- one ring would
        # arrival-pace the whole prologue. All slices are contiguous per
        # partition. First halves (cols 0:1024 = lg 0-1) of xk/xq go first.
        # the first few chunks are partition-SPLIT into halves: each
        # dma_start lands on a different HW queue, halving arrival latency
        # for the prologue-critical data
        # prologue-critical transfers are partition-split: a [128, dc, 1024]
        # chunk is ~128 descriptors processed serially on ONE queue (~5-7us
        # latency); quarters/halves land on separate queues and arrive 2-4x
        # sooner. Later chunks stay whole (PE is behind by then anyway).
        bias_sb = bias_pool.tile([128, 4], fp32, name="bqk")
        nc.scalar.dma_start(out=bias_sb, in_=bqk[:, :])
        for ph in range(2):
            p = slice(ph * 64, ph * 64 + 64)
            nc.scalar.dma_start(out=wq_sb[0][p, 0:2, :], in_=wq_r[p, 0:2, :])
        for dc in range(2):
            for ph in range(4):
                p = slice(ph * 32, ph * 32 + 32)
                nc.scalar.dma_start(out=xq_sb[p, dc, 0:1024], in_=xq_r[p, dc, 0:1024])
        for ph in range(2):
            p = slice(ph * 64, ph * 64 + 64)
            nc.scalar.dma_start(out=wq_sb[0][p, 2:4, :], in_=wq_r[p, 2:4, :])
        for dc in range(2, 4):
            for ph in range(2):
                p = slice(ph * 64, ph * 64 + 64)
                nc.scalar.dma_start(out=xq_sb[p, dc, 0:1024], in_=xq_r[p, dc, 0:1024])
        nc.scalar.dma_start(out=wq_sb[1], in_=wq_r[:, 4:8, :])
        for dc in range(4, 8):
            nc.scalar.dma_start(out=xq_sb[:, dc, 0:1024], in_=xq_r[:, dc, 0:1024])
        for ph in range(2):
            p = slice(ph * 64, ph * 64 + 64)
            nc.sync.dma_start(out=wk_sb[0][p, 0:2, :], in_=wk_r[p, 0:2, :])
        for dc in range(2):
            for ph in range(4):
                p = slice(ph * 32, ph * 32 + 32)
                nc.sync.dma_start(out=xk_sb[p, dc, 0:1024], in_=xk_r[p, dc, 0:1024])
        for ph in range(2):
            p = slice(ph * 64, ph * 64 + 64)
            nc.sync.dma_start(out=wk_sb[0][p, 2:4, :], in_=wk_r[p, 2:4, :])
        for dc in range(2, 4):
            for ph in range(2):
                p = slice(ph * 64, ph * 64 + 64)
                nc.sync.dma_start(out=xk_sb[p, dc, 0:1024], in_=xk_r[p, dc, 0:1024])
        nc.sync.dma_start(out=wk_sb[1], in_=wk_r[:, 4:8, :])
        for dc in range(4, 8):
            nc.sync.dma_start(out=xk_sb[:, dc, 0:1024], in_=xk_r[:, dc, 0:1024])
        nc.sync.dma_start(out=wv_sb, in_=wv_r)
        for g in range(4):
            nc.sync.dma_start(
                out=xv_sb[g].rearrange("p dc c -> p (dc c)"),
                in_=xv_r[:, g].rearrange("p dc c -> p (dc c)"),
            )
        wo_sb = []
        for cc in range(2):
            t = wo_pool.tile([128, D], mm_dt, name=f"wo{cc}")
            nc.sync.dma_start(out=t, in_=wo[cc * 128:(cc + 1) * 128, :])
            wo_sb.append(t)
        for dc in range(8):
            nc.sync.dma_start(out=xk_sb[:, dc, 1024:2048], in_=xk_r[:, dc, 1024:2048])
            nc.scalar.dma_start(out=xq_sb[:, dc, 1024:2048], in_=xq_r[:, dc, 1024:2048])

        # ---- persistent SBUF staging ------------------------------------
        qt_sb = [qt_pools[i].tile([128, L], mm_dt, name=f"qt{i}") for i in range(2)]
        kt_sb = [kt_pools[i].tile([128, L], mm_dt, name=f"kt{i}") for i in range(2)]
        # V natural layout, one tile per lt-quad: [128, 4 lt, 4 head, 65]
        # (col 64 = ones)
        v_sb = [vn_pools[g].tile([128, 4, 4, 65], mm_dt, name=f"v{g}")
                for g in range(4)]
        for g in range(4):
            nc.vector.memset(v_sb[g][:, :, :, 64:65], 1.0)
        # one-time [128,128] causal triangle (tri[p,f] = f >= p); applied on
        # GPSIMD via tensor_mul (SBUF-only; gpsimd has no PSUM port)
        tri_sb = bias_pool.tile([128, 128], mm_dt, name="tri")
        nc.vector.memset(tri_sb, 1.0)
        nc.gpsimd.affine_select(
            out=tri_sb,
            in_=tri_sb,
            compare_op=mybir.AluOpType.is_ge,
            fill=0.0,
            base=0,
            channel_multiplier=-1,
            pattern=[[1, 128]],
        )
        # per-(cc, g4) O tiles in per-g4 POOLS (see pool note above)
        ot_sb = [[ot_pools[g].tile([128, 512], mm_dt, name=f"ot{i}g{g}")
                  for g in range(4)] for i in range(2)]
        y_view = y.rearrange("(lt p) c -> p lt c", p=128)

        # ---- filler units (popped into PE stall windows) ---------------
        def qk_units(dst, x_sb, w_ab, bidx, lg, cc):
            state = {}

            def mk(i):
                def f():
                    if i == 0:
                        state["ps"] = psA.tile([128, 512], fp32, name="ps")
                    ps = state["ps"]
                    for dc in (2 * i, 2 * i + 1):
                        nc.tensor.matmul(
                            ps,
                            w_ab[dc // 4][:, dc % 4, cc * 128:(cc + 1) * 128],
                            x_sb[:, dc, lg * 512:(lg + 1) * 512],
                            start=(dc == 0),
                            stop=(dc == 7),
                        )
                    if i == 3:
                        nc.vector.tensor_scalar_add(
                            out=dst[cc][:, lg * 512:(lg + 1) * 512],
                            in0=ps,
                            scalar1=bias_sb[:, bidx:bidx + 1],
                        )
                return f

            return [(("qk", lg, cc), 900, mk(i)) for i in range(4)]

        def v_units(lt):
            state = {}

            def mk(i):
                def f():
                    if i == 0:
                        state["ps"] = psA.tile([128, CPC], fp32, name="ps")
                    ps = state["ps"]
                    for dc in (2 * i, 2 * i + 1):
                        nc.tensor.matmul(
                            ps,
                            xv_sb[lt // 4][:, dc, (lt % 4) * 128:(lt % 4 + 1) * 128],
                            wv_sb[:, dc, :],
                            start=(dc == 0),
                            stop=(dc == 7),
                        )
                    if i == 3:
                        nc.vector.tensor_copy(
                            out=v_sb[lt // 4][:, lt % 4, :, 0:64],
                            in_=ps.rearrange("p (h d) -> p h d", d=64),
                        )
                return f

            return [(("v", lt), 450, mk(i)) for i in range(4)]

        def c_units(g4, act_copy=False):
            units = []
            for li in range(4):
                lt = g4 * 4 + li
                for dg in range(2):
                    def f(lt=lt, dg=dg):
                        ps = psA.tile([128, 512], fp32, name="ps")
                        for cc2 in range(2):
                            nc.tensor.matmul(
                                ps,
                                ot_sb[cc2][lt // 4][:, (lt % 4) * 128:
                                                    (lt % 4 + 1) * 128],
                                wo_sb[cc2][:, dg * 512:(dg + 1) * 512],
                                start=(cc2 == 0),
                                stop=(cc2 == 1),
                            )
                        yt = y_pool.tile([128, 512], mm_dt, name="yh")
                        if act_copy:
                            # tail units: copy on ACT (idle once exps done)
                            # and DMA from ACT too -- the sync ring is
                            # blocked in-order on earlier yt semaphores at
                            # that point and would defer the issue by >10us
                            nc.scalar.activation(
                                out=yt, in_=ps,
                                func=mybir.ActivationFunctionType.Copy,
                                bias=0.0,
                            )
                            nc.scalar.dma_start(
                                out=y_view[:, lt, dg * 512:(dg + 1) * 512],
                                in_=yt,
                            )
                        else:
                            nc.vector.tensor_copy(out=yt, in_=ps)
                            nc.sync.dma_start(
                                out=y_view[:, lt, dg * 512:(dg + 1) * 512],
                                in_=yt,
                            )
                    units.append((("c", g4), 500, f))
            return units

        queue = []
        reserve = []

        def pump_ns(budget):
            while budget > 0 and queue:
                _, cost, f = queue.pop(0)
                f()
                budget -= cost

        def need(tag):
            # emit from the front until no unit with this tag remains
            while any(t == tag for t, _, _ in queue):
                queue.pop(0)[2]()

        # ---- prologue: ALL lg=0 projections serially, cc0/cc1 interleaved
        # per dc-pair. Both cc's units consume the same xk/xq chunk, so the
        # interleave gives the PE 2x work per DMA arrival instead of idling
        # at chunk boundaries (the K ring is on sync, Q on ACT -- they load
        # in parallel).
        k0 = qk_units(kt_sb, xk_sb, wk_sb, 2, 0, 0)
        k1 = qk_units(kt_sb, xk_sb, wk_sb, 3, 0, 1)
        q0 = qk_units(qt_sb, xq_sb, wq_sb, 0, 0, 0)
        q1 = qk_units(qt_sb, xq_sb, wq_sb, 1, 0, 1)
        for i in range(4):
            k0[i][2]()
            k1[i][2]()
        for i in range(4):
            q0[i][2]()
            q1[i][2]()
        queue += v_units(0) + v_units(1) + v_units(2) + v_units(3)

        EXP = mybir.ActivationFunctionType.Exp

        # ---- per-key-tile S + exp. psS holds TWO [128,1024] tiles so the
        # next S never waits for the previous exp to drain (the baseline's
        # single 4-bank tile serialized S<->EXP at ~1.3us per key-pair)
        def emit_S(g4, cc, kt):
            diag = (kt // 4 == g4)
            off = 128 * (kt % 4) if diag else 0
            st = psS.tile([128, 1024], fp32, name="st")
            pt = pt_pool.tile([128, 1024], mm_dt, name="pt")
            for idx in range(2):
                r = idx * 64
                nc.tensor.matmul(
                    st[:, idx * 512 + off:(idx + 1) * 512],
                    kt_sb[cc][r:r + 64, kt * 128:(kt + 1) * 128],
                    qt_sb[cc][r:r + 64, g4 * 512 + off:(g4 + 1) * 512],
                    start=True,
                    stop=True,
                )
            if not diag:
                nc.scalar.activation(out=pt, in_=st, func=EXP, scale=0.125)
            else:
                st3 = st.rearrange("p (i c) -> p i c", i=2)
                pt3 = pt.rearrange("p (i c) -> p i c", i=2)
                nc.scalar.activation(
                    out=pt3[:, :, off:512],
                    in_=st3[:, :, off:512],
                    func=EXP,
                    scale=0.125,
                )
                # mask only the 128-wide staircase strip (on DVE: gpsimd
                # tensor ops are slower and force ucode library reloads
                # around the partition_broadcasts)
                strip = pt3[:, :, off:off + 128]
                nc.vector.tensor_mul(
                    out=strip,
                    in0=strip,
                    in1=tri_sb[:, None, :].broadcast_to([128, 2, 128]),
                )
            return pt

        def emit_PV(g4, hp, kt, pt, ot_ps, nkt):
            diag = (kt // 4 == g4)
            off = 128 * (kt % 4) if diag else 0
            for idx in range(2):
                h = 2 * hp + idx
                if g4 == kt // 4:
                    need(("v", kt))
                nc.tensor.matmul(
                    ot_ps[idx][:, off:512],
                    v_sb[kt // 4][:, kt % 4, h, :],
                    pt[:, idx * 512 + off:(idx + 1) * 512],
                    start=(kt == 0),
                    stop=(kt == nkt - 1),
                )

        PACE = [700, 600, 350, 300]

        for g4 in range(4):
            # refill the filler queue (ordered by first-need time). C(g) is
            # queued at g4=g+1 so the 4MB y writeback spreads over the whole
            # run instead of jamming the DMA queues in the last 13us.
            if g4 == 0:
                queue += qk_units(kt_sb, xk_sb, wk_sb, 2, 1, 0)
                queue += qk_units(kt_sb, xk_sb, wk_sb, 3, 1, 1)
                queue += qk_units(qt_sb, xq_sb, wq_sb, 0, 1, 0)
                queue += qk_units(qt_sb, xq_sb, wq_sb, 1, 1, 1)
            elif g4 == 1:
                queue += v_units(4) + v_units(5) + v_units(6) + v_units(7)
                queue += qk_units(kt_sb, xk_sb, wk_sb, 2, 2, 0)
                queue += qk_units(kt_sb, xk_sb, wk_sb, 3, 2, 1)
                queue += qk_units(qt_sb, xq_sb, wq_sb, 0, 2, 0)
                queue += qk_units(qt_sb, xq_sb, wq_sb, 1, 2, 1)
                queue += c_units(0)
            elif g4 == 2:
                queue += v_units(8) + v_units(9) + v_units(10) + v_units(11)
                queue += qk_units(kt_sb, xk_sb, wk_sb, 2, 3, 0)
                queue += qk_units(kt_sb, xk_sb, wk_sb, 3, 3, 1)
                queue += qk_units(qt_sb, xq_sb, wq_sb, 0, 3, 0)
                queue += qk_units(qt_sb, xq_sb, wq_sb, 1, 3, 1)
                queue += c_units(1)
            elif g4 == 3:
                queue += v_units(12) + v_units(13) + v_units(14) + v_units(15)
                queue += c_units(2)[:4]
                reserve.extend(c_units(2, act_copy=True)[4:])

            nkt = 4 * g4 + 4
            for hp in range(2):
                cc = hp
                need(("qk", g4, cc))
                ot_ps = [
                    psOT.tile([65, 512], fp32, name="ot0"),
                    psOT.tile([65, 512], fp32, name="ot1"),
                ]
                prev = None
                for kt in range(nkt):
                    cur = emit_S(g4, cc, kt)
                    # batched pumping: fewer PSUM-bank regime switches on
                    # the PE (HAM K-state cycles on micro-idles between
                    # mm groups)
                    if kt % 2 == 1:
                        pump_ns(2 * PACE[g4])
                    if kt > 0:
                        emit_PV(g4, hp, kt - 1, prev, ot_ps, nkt)
                    prev = cur
                # last key tile: emit each head's PV then immediately its
                # division so the DVE chain starts earlier. No pumping
                # between division emissions (transitive FIFO stalls).
                kt = nkt - 1
                diag_l = (kt // 4 == g4)
                off = 128 * (kt % 4) if diag_l else 0
                if g4 == kt // 4:
                    need(("v", kt))
                for idx in range(2):
                    h = 2 * hp + idx
                    nc.tensor.matmul(
                        ot_ps[idx][:, off:512],
                        v_sb[kt // 4][:, kt % 4, h, :],
                        prev[:, idx * 512 + off:(idx + 1) * 512],
                        start=(kt == 0),
                        stop=True,
                    )
                # softmax division, latency-interleaved across engines:
                # z row copy on ACT (PSUM->SBUF; ACT is idle right here),
                # recip on the [1,512] row (DVE), THEN broadcast the
                # reciprocal (GPSIMD), mul on DVE.
                # NB: reciprocal reading PSUM directly is silently wrong.
                zrow = [None, None]
                zrc = [None, None]
                zbr = [None, None]
                for idx in range(2):
                    zrow[idx] = zr_pool.tile([1, 512], fp32, name=f"zrow{idx}")
                    nc.scalar.activation(
                        out=zrow[idx], in_=ot_ps[idx][64:65, :],
                        func=mybir.ActivationFunctionType.Copy, bias=0.0,
                    )
                    zrc[idx] = zc_pool.tile([1, 512], fp32, name=f"zrc{idx}")
                    nc.vector.reciprocal_approx_fast(out=zrc[idx], in_=zrow[idx])
                for idx in range(2):
                    zbr[idx] = zb_pool.tile([64, 512], fp32, name=f"zbr{idx}")
                    nc.gpsimd.partition_broadcast(out_ap=zbr[idx], in_ap=zrc[idx])
                if (g4, hp) != (3, 1):
                    for idx in range(2):
                        nc.vector.tensor_mul(
                            out=ot_sb[cc][g4][idx * 64:idx * 64 + 64, :],
                            in0=ot_ps[idx][0:64, :],
                            in1=zbr[idx],
                        )
                    pump_ns(3000)
                else:
                    # final divisions: muls deferred to the tail, split per
                    # 128-col q-block and interleaved with the C(3) cc1
                    # matmuls so each C group starts as soon as ITS block
                    # is divided
                    last_ot_ps, last_zbr = ot_ps, zbr

        # ---- tail: reserved C units bridge the last division window; then
        # C(3) in [128,1024] psS groups (double-buffered)
        for _, _, f in reserve:
            f()
        while queue:
            queue.pop(0)[2]()
        # C(3): the cc2=0 halves only need hp=0's divisions (done ~2.5us
        # before hp=1's), so they are the ONLY work that becomes ready
        # DURING the final division window -- emit them first so the PE has
        # something to chew on while DVE runs the last chain. lt 12/13 in
        # the two psS slots, lt 14 in the two psA slots.
        st3 = {}
        for lt in (12, 13):
            st3[lt] = psS.tile([128, 1024], fp32, name="st")
            for dg in range(2):
                nc.tensor.matmul(
                    st3[lt][:, dg * 512:(dg + 1) * 512],
                    ot_sb[0][3][:, (lt % 4) * 128:(lt % 4 + 1) * 128],
                    wo_sb[0][:, dg * 512:(dg + 1) * 512],
                    start=True, stop=False,
                )
        for dg in range(2):
            st3[(14, dg)] = psA.tile([128, 512], fp32, name="ps")
            nc.tensor.matmul(
                st3[(14, dg)],
                ot_sb[0][3][:, 2 * 128:3 * 128],
                wo_sb[0][:, dg * 512:(dg + 1) * 512],
                start=True, stop=False,
            )
        # per-block division muls interleaved with the cc1 C matmuls
        for blk in range(4):
            bs = slice(blk * 128, (blk + 1) * 128)
            for idx in range(2):
                nc.vector.tensor_mul(
                    out=ot_sb[1][3][idx * 64:idx * 64 + 64, bs],
                    in0=last_ot_ps[idx][0:64, bs],
                    in1=last_zbr[idx][:, bs],
                )
            lt = 12 + blk
            if lt == 15:
                st15 = psS.tile([128, 1024], fp32, name="st")
                for dg in range(2):
                    for cc2 in range(2):
                        nc.tensor.matmul(
                            st15[:, dg * 512:(dg + 1) * 512],
                            ot_sb[cc2][3][:, bs],
                            wo_sb[cc2][:, dg * 512:(dg + 1) * 512],
                            start=(cc2 == 0),
                            stop=(cc2 == 1),
                        )
                yt = y_pool.tile([128, 1024], mm_dt, name="yt")
                nc.vector.tensor_copy(out=yt[:, 0:512], in_=st15[:, 0:512])
                nc.scalar.activation(
                    out=yt[:, 512:1024], in_=st15[:, 512:1024],
                    func=mybir.ActivationFunctionType.Copy,
                    bias=0.0,
                )
                nc.scalar.dma_start(out=y_view[:, 15, :], in_=yt)
            elif lt == 14:
                for dg in range(2):
                    nc.tensor.matmul(
                        st3[(14, dg)],
                        ot_sb[1][3][:, bs],
                        wo_sb[1][:, dg * 512:(dg + 1) * 512],
                        start=False, stop=True,
                    )
                    yt = y_pool.tile([128, 512], mm_dt, name="yh")
                    if dg == 0:
                        nc.vector.tensor_copy(out=yt, in_=st3[(14, dg)])
                    else:
                        nc.scalar.activation(
                            out=yt, in_=st3[(14, dg)],
                            func=mybir.ActivationFunctionType.Copy, bias=0.0,
                        )
                    nc.scalar.dma_start(
                        out=y_view[:, 14, dg * 512:(dg + 1) * 512], in_=yt)
            else:
                for dg in range(2):
                    nc.tensor.matmul(
                        st3[lt][:, dg * 512:(dg + 1) * 512],
                        ot_sb[1][3][:, bs],
                        wo_sb[1][:, dg * 512:(dg + 1) * 512],
                        start=False, stop=True,
                    )
                yt = y_pool.tile([128, 1024], mm_dt, name="yt")
                nc.vector.tensor_copy(out=yt[:, 0:512], in_=st3[lt][:, 0:512])
                nc.scalar.activation(
                    out=yt[:, 512:1024], in_=st3[lt][:, 512:1024],
                    func=mybir.ActivationFunctionType.Copy,
                    bias=0.0,
                )
                nc.scalar.dma_start(out=y_view[:, lt, :], in_=yt)

    nc.compile()
    return nc


def _get_nc(mm_dt: str):
    if mm_dt not in _CACHE:
        _CACHE[mm_dt] = build_nc(mm_dt)
    return _CACHE[mm_dt]


def kernel(q, k, v, mask, Wq, bq, Wk, bk, Wv, bv, Wo, bo, _trace=False):
    nc = _get_nc(MM_DT)

    # DMA-friendly host swizzles (contiguous per partition on-device):
    #  x  [D,L] -> [128, dc*L]            A[p, dc*L+c]       = xT[dc*128+p, c]
    #  xv [D,L] -> [128, g*dc*512]        A[p,(g*8+dc)*512+c] = xT[dc*128+p, g*512+c]
    #  w  [D,CPC] -> [128, dc*CPC]        A[p, dc*CPC+c]     = W[dc*128+p, c]
    def _sw_x(xt):
        return np.ascontiguousarray(
            xt.reshape(8, 128, L).transpose(1, 0, 2).reshape(128, 8 * L))

    def _sw_xv(xt):
        return np.ascontiguousarray(
            xt.reshape(8, 128, 4, 512).transpose(1, 2, 0, 3).reshape(128, 4 * 8 * 512))

    def _sw_w(w):
        return np.ascontiguousarray(
            w.reshape(8, 128, CPC).transpose(1, 0, 2).reshape(128, 8 * CPC))

    in_maps = []
    for c in range(NCORES):
        b = c // 4
        g = c % 4
        s = slice(g * CPC, (g + 1) * CPC)
        bq_s = np.ascontiguousarray(bq[s]).reshape(2, 128).T
        bk_s = np.ascontiguousarray(bk[s]).reshape(2, 128).T
        qT = np.ascontiguousarray(q[b].T).astype(NP_MM)
        kT = np.ascontiguousarray(k[b].T).astype(NP_MM)
        vT = np.ascontiguousarray(v[b].T).astype(NP_MM)
        in_maps.append({
            "xq": _sw_x(qT),
            "xk": _sw_x(kT),
            "xv": _sw_xv(vT),
            "wq": _sw_w(np.ascontiguousarray(Wq[:, s]).astype(NP_MM)),
            "wk": _sw_w(np.ascontiguousarray(Wk[:, s]).astype(NP_MM)),
            "wv": _sw_w(np.ascontiguousarray(Wv[:, s]).astype(NP_MM)),
            "wo": np.ascontiguousarray(Wo[s, :]).astype(NP_MM),
            "bqk": np.ascontiguousarray(
                np.concatenate([bq_s, bk_s], axis=1)).astype(np.float32),
        })

    res = run_bass_kernel_spmd(nc, in_maps, list(range(NCORES)), trace=_trace)

    # host gather: out[b] = sum_g y_core(b,g) + (bo + bv @ Wo)
    const = (bo + bv.astype(np.float64) @ Wo.astype(np.float64)).astype(np.float64)
    out = np.zeros((B, L, D), np.float64)
    for c in range(NCORES):
        out[c // 4] += res.results[c]["y"].astype(np.float64)
    out += const[None, None, :]
    kernel.last_exec_time_ns = res.exec_time_ns
    return out.astype(np.float32)


# revision 34
# speedup vs baseline: 1.0436x; 1.0436x over previous
import sys

sys.path.insert(0, "/opt/trn_rl_repo")

import numpy as np
import ml_dtypes

import concourse.bass as bass
import concourse.bacc as bacc
import concourse.tile as tile
from concourse.bass_utils import run_bass_kernel_spmd
from concourse import mybir

B, L, D, H = 2, 2048, 1024, 16
DH = 64          # dim per head
HPC = 4          # heads per core
CPC = HPC * DH   # feature cols per core = 256
NCORES = 8

MM_DT = "bfloat16"
NP_MM = ml_dtypes.bfloat16 if MM_DT == "bfloat16" else np.float32

_CACHE = {}


def build_nc(mm_dt: str):
    nc = bacc.Bacc()
    mm_dt = mybir.dt(mm_dt)
    fp32 = mybir.dt.float32

    # host pre-swizzles every input so each DMA is CONTIGUOUS per partition
    # (sync-engine DIRECT2D descriptor-gen cost scales with line count; the
    # naive "(dc p) c -> p dc c" patterns cost 3-4us of sync time EACH and
    # stall every semaphore relay queued behind them)
    xq = nc.declare_dram_parameter("xq", (128, 8 * L), mm_dt, isOutput=False)
    xk = nc.declare_dram_parameter("xk", (128, 8 * L), mm_dt, isOutput=False)
    xv = nc.declare_dram_parameter("xv", (128, 4 * 8 * 512), mm_dt, isOutput=False)
    wq = nc.declare_dram_parameter("wq", (128, 8 * CPC), mm_dt, isOutput=False)
    wk = nc.declare_dram_parameter("wk", (128, 8 * CPC), mm_dt, isOutput=False)
    wv = nc.declare_dram_parameter("wv", (128, 8 * CPC), mm_dt, isOutput=False)
    wo = nc.declare_dram_parameter("wo", (CPC, D), mm_dt, isOutput=False)
    # packed biases: cols 0:2 = bq (cc0,cc1), cols 2:4 = bk -- single fat
    # descriptor instead of 256 4-byte ones
    bqk = nc.declare_dram_parameter("bqk", (128, 4), fp32, isOutput=False)
    y = nc.declare_dram_parameter("y", (L, D), mm_dt, isOutput=True)      # partial out (bf16)

    from contextlib import ExitStack

    with ExitStack() as es:
        tc = es.enter_context(tile.TileContext(nc))
        # NOTE: bufs are per named tag
        xt_pool = es.enter_context(tc.tile_pool(name="xt", bufs=1))     # xq/xk [128,8,2048]
        xv_pool = es.enter_context(tc.tile_pool(name="xv", bufs=1))     # 4 tags [128,8,512]
        w_pool = es.enter_context(tc.tile_pool(name="w", bufs=1))       # wk/wq halves + wv
        wo_pool = es.enter_context(tc.tile_pool(name="wo", bufs=1))     # 2 tags [128,1024]
        bias_pool = es.enter_context(tc.tile_pool(name="bias", bufs=1))
        # NOTE: matmul weight-reads (ldweights) get a conservative sync dep
        # on the LAST write to the same pool at emission time, not the
        # precise slice. Separate pools per consumer group so e.g. the
        # C(2) fillers never wait on g4=3's divisions.
        qt_pools = [es.enter_context(tc.tile_pool(name=f"qt{i}", bufs=1))
                    for i in range(2)]
        kt_pools = [es.enter_context(tc.tile_pool(name=f"kt{i}", bufs=1))
                    for i in range(2)]
        vn_pools = [es.enter_context(tc.tile_pool(name=f"vn{g}", bufs=1))
                    for g in range(4)]
        pt_pool = es.enter_context(tc.tile_pool(name="pt", bufs=3))     # [128,1024] bf16
        zr_pool = es.enter_context(tc.tile_pool(name="zr", bufs=2))     # [1,512] z rows
        zb_pool = es.enter_context(tc.tile_pool(name="zb", bufs=2))     # [64,512] bcast
        zc_pool = es.enter_context(tc.tile_pool(name="zc", bufs=2))     # [64,512] recip
        ot_pools = [es.enter_context(tc.tile_pool(name=f"otp{g}", bufs=1))
                    for g in range(4)]                                  # 2 tags [128,512] each
        y_pool = es.enter_context(tc.tile_pool(name="ysb", bufs=8))     # yh [128,512] / yt [128,1024]
        psA = es.enter_context(tc.tile_pool(name="psA", bufs=2, space="PSUM"))   # 2 banks
        psS = es.enter_context(tc.tile_pool(name="psS", bufs=2, space="PSUM"))   # [128,1024] x2 = 4 banks
        psOT = es.enter_context(tc.tile_pool(name="psOT", bufs=1, space="PSUM"))  # 2 tags [65,512] = 2 banks

        # ---- SBUF staging tiles ----------------------------------------
        # w tiles split into dc-halves: qk-unit weight reads are
        # conservative whole-tile deps, so a single [128,8,CPC] tile makes
        # the FIRST matmul wait for the LAST wk DMA (~2.4us of dead start)
        wk_sb = [w_pool.tile([128, 4, CPC], mm_dt, name=f"wk{h}") for h in range(2)]
        wq_sb = [w_pool.tile([128, 4, CPC], mm_dt, name=f"wq{h}") for h in range(2)]
        wv_sb = w_pool.tile([128, 8, CPC], mm_dt, name="wv")   # moving reads: precise
        xk_sb = xt_pool.tile([128, 8, L], mm_dt, name="xk")
        xq_sb = xt_pool.tile([128, 8, L], mm_dt, name="xq")
        # xv is the STATIONARY operand of v-units -> per-lg tiles so v(0..3)
        # only wait on the first xv chunk
        xv_sb = [xv_pool.tile([128, 8, 512], mm_dt, name=f"xv{g}") for g in range(4)]

        wk_r = wk.rearrange("p (dc c) -> p dc c", dc=8)
        wq_r = wq.rearrange("p (dc c) -> p dc c", dc=8)
        wv_r = wv.rearrange("p (dc c) -> p dc c", dc=8)
        xk_r = xk.rearrange("p (dc c) -> p dc c", dc=8)
        xq_r = xq.rearrange("p (dc c) -> p dc c", dc=8)
        xv_r = xv.rearrange("p (g dc c) -> p g dc c", g=4, dc=8)

        # TWO DMA issue rings: the K-side on sync (SP), the Q-side on ACT
        # (idle until the first exp at ~28us). Descriptor gen (DIRECT2D) is
        # ~0.6us per dma_start and serializes per engine -- one ring would
        # arrival-pace the whole prologue. All slices are contiguous per
        # partition. First halves (cols 0:1024 = lg 0-1) of xk/xq go first.
        # the first few chunks are partition-SPLIT into halves: each
        # dma_start lands on a different HW queue, halving arrival latency
        # for the prologue-critical data
        bias_sb = bias_pool.tile([128, 4], fp32, name="bqk")
        nc.scalar.dma_start(out=bias_sb, in_=bqk[:, :])
        nc.scalar.dma_start(out=wq_sb[0][:, 0:2, :], in_=wq_r[:, 0:2, :])
        nc.scalar.dma_start(out=xq_sb[:, 0, 0:1024], in_=xq_r[:, 0, 0:1024])
        nc.scalar.dma_start(out=xq_sb[:, 1, 0:1024], in_=xq_r[:, 1, 0:1024])
        nc.scalar.dma_start(out=wq_sb[0][:, 2:4, :], in_=wq_r[:, 2:4, :])
        nc.scalar.dma_start(out=wq_sb[1], in_=wq_r[:, 4:8, :])
        for dc in range(2, 8):
            nc.scalar.dma_start(out=xq_sb[:, dc, 0:1024], in_=xq_r[:, dc, 0:1024])
        nc.sync.dma_start(out=wk_sb[0][:, 0:2, :], in_=wk_r[:, 0:2, :])
        nc.sync.dma_start(out=xk_sb[:, 0, 0:1024], in_=xk_r[:, 0, 0:1024])
        nc.sync.dma_start(out=xk_sb[:, 1, 0:1024], in_=xk_r[:, 1, 0:1024])
        nc.sync.dma_start(out=wk_sb[0][:, 2:4, :], in_=wk_r[:, 2:4, :])
        nc.sync.dma_start(out=wk_sb[1], in_=wk_r[:, 4:8, :])
        for dc in range(2, 8):
            nc.sync.dma_start(out=xk_sb[:, dc, 0:1024], in_=xk_r[:, dc, 0:1024])
        nc.sync.dma_start(out=wv_sb, in_=wv_r)
        for g in range(4):
            nc.sync.dma_start(
                out=xv_sb[g].rearrange("p dc c -> p (dc c)"),
                in_=xv_r[:, g].rearrange("p dc c -> p (dc c)"),
            )
        wo_sb = []
        for cc in range(2):
            t = wo_pool.tile([128, D], mm_dt, name=f"wo{cc}")
            nc.sync.dma_start(out=t, in_=wo[cc * 128:(cc + 1) * 128, :])
            wo_sb.append(t)
        for dc in range(8):
            nc.sync.dma_start(out=xk_sb[:, dc, 1024:2048], in_=xk_r[:, dc, 1024:2048])
            nc.scalar.dma_start(out=xq_sb[:, dc, 1024:2048], in_=xq_r[:, dc, 1024:2048])

        # ---- persistent SBUF staging ------------------------------------
        qt_sb = [qt_pools[i].tile([128, L], mm_dt, name=f"qt{i}") for i in range(2)]
        kt_sb = [kt_pools[i].tile([128, L], mm_dt, name=f"kt{i}") for i in range(2)]
        # V natural layout, one tile per lt-quad: [128, 4 lt, 4 head, 65]
        # (col 64 = ones)
        v_sb = [vn_pools[g].tile([128, 4, 4, 65], mm_dt, name=f"v{g}")
                for g in range(4)]
        for g in range(4):
            nc.vector.memset(v_sb[g][:, :, :, 64:65], 1.0)
        # one-time [128,128] causal triangle (tri[p,f] = f >= p); applied on
        # GPSIMD via tensor_mul (SBUF-only; gpsimd has no PSUM port)
        tri_sb = bias_pool.tile([128, 128], mm_dt, name="tri")
        nc.vector.memset(tri_sb, 1.0)
        nc.gpsimd.affine_select(
            out=tri_sb,
            in_=tri_sb,
            compare_op=mybir.AluOpType.is_ge,
            fill=0.0,
            base=0,
            channel_multiplier=-1,
            pattern=[[1, 128]],
        )
        # per-(cc, g4) O tiles in per-g4 POOLS (see pool note above)
        ot_sb = [[ot_pools[g].tile([128, 512], mm_dt, name=f"ot{i}g{g}")
                  for g in range(4)] for i in range(2)]
        y_view = y.rearrange("(lt p) c -> p lt c", p=128)

        # ---- filler units (popped into PE stall windows) ---------------
        def qk_units(dst, x_sb, w_ab, bidx, lg, cc):
            state = {}

            def mk(i):
                def f():
                    if i == 0:
                        state["ps"] = psA.tile([128, 512], fp32, name="ps")
                    ps = state["ps"]
                    for dc in (2 * i, 2 * i + 1):
                        nc.tensor.matmul(
                            ps,
                            w_ab[dc // 4][:, dc % 4, cc * 128:(cc + 1) * 128],
                            x_sb[:, dc, lg * 512:(lg + 1) * 512],
                            start=(dc == 0),
                            stop=(dc == 7),
                        )
                    if i == 3:
                        nc.vector.tensor_scalar_add(
                            out=dst[cc][:, lg * 512:(lg + 1) * 512],
                            in0=ps,
                            scalar1=bias_sb[:, bidx:bidx + 1],
                        )
                return f

            return [(("qk", lg, cc), 900, mk(i)) for i in range(4)]

        def v_units(lt):
            state = {}

            def mk(i):
                def f():
                    if i == 0:
                        state["ps"] = psA.tile([128, CPC], fp32, name="ps")
                    ps = state["ps"]
                    for dc in (2 * i, 2 * i + 1):
                        nc.tensor.matmul(
                            ps,
                            xv_sb[lt // 4][:, dc, (lt % 4) * 128:(lt % 4 + 1) * 128],
                            wv_sb[:, dc, :],
                            start=(dc == 0),
                            stop=(dc == 7),
                        )
                    if i == 3:
                        nc.vector.tensor_copy(
                            out=v_sb[lt // 4][:, lt % 4, :, 0:64],
                            in_=ps.rearrange("p (h d) -> p h d", d=64),
                        )
                return f

            return [(("v", lt), 450, mk(i)) for i in range(4)]

        def c_units(g4, act_copy=False):
            units = []
            for li in range(4):
                lt = g4 * 4 + li
                for dg in range(2):
                    def f(lt=lt, dg=dg):
                        ps = psA.tile([128, 512], fp32, name="ps")
                        for cc2 in range(2):
                            nc.tensor.matmul(
                                ps,
                                ot_sb[cc2][lt // 4][:, (lt % 4) * 128:
                                                    (lt % 4 + 1) * 128],
                                wo_sb[cc2][:, dg * 512:(dg + 1) * 512],
                                start=(cc2 == 0),
                                stop=(cc2 == 1),
                            )
                        yt = y_pool.tile([128, 512], mm_dt, name="yh")
                        if act_copy:
                            # tail units: copy on ACT (idle once exps done)
                            # and DMA from ACT too -- the sync ring is
                            # blocked in-order on earlier yt semaphores at
                            # that point and would defer the issue by >10us
                            nc.scalar.activation(
                                out=yt, in_=ps,
                                func=mybir.ActivationFunctionType.Copy,
                                bias=0.0,
                            )
                            nc.scalar.dma_start(
                                out=y_view[:, lt, dg * 512:(dg + 1) * 512],
                                in_=yt,
                            )
                        else:
                            nc.vector.tensor_copy(out=yt, in_=ps)
                            nc.sync.dma_start(
                                out=y_view[:, lt, dg * 512:(dg + 1) * 512],
                                in_=yt,
                            )
                    units.append((("c", g4), 500, f))
            return units

        queue = []
        reserve = []

        def pump_ns(budget):
            while budget > 0 and queue:
                _, cost, f = queue.pop(0)
                f()
                budget -= cost

        def need(tag):
            # emit from the front until no unit with this tag remains
            while any(t == tag for t, _, _ in queue):
                queue.pop(0)[2]()

        # ---- prologue: ALL lg=0 projections serially, cc0/cc1 interleaved
        # per dc-pair. Both cc's units consume the same xk/xq chunk, so the
        # interleave gives the PE 2x work per DMA arrival instead of idling
        # at chunk boundaries (the K ring is on sync, Q on ACT -- they load
        # in parallel).
        k0 = qk_units(kt_sb, xk_sb, wk_sb, 2, 0, 0)
        k1 = qk_units(kt_sb, xk_sb, wk_sb, 3, 0, 1)
        q0 = qk_units(qt_sb, xq_sb, wq_sb, 0, 0, 0)
        q1 = qk_units(qt_sb, xq_sb, wq_sb, 1, 0, 1)
        for i in range(4):
            k0[i][2]()
            k1[i][2]()
        for i in range(4):
            q0[i][2]()
            q1[i][2]()
        queue += v_units(0) + v_units(1) + v_units(2) + v_units(3)

        EXP = mybir.ActivationFunctionType.Exp

        # ---- per-key-tile S + exp. psS holds TWO [128,1024] tiles so the
        # next S never waits for the previous exp to drain (the baseline's
        # single 4-bank tile serialized S<->EXP at ~1.3us per key-pair)
        def emit_S(g4, cc, kt):
            diag = (kt // 4 == g4)
            off = 128 * (kt % 4) if diag else 0
            st = psS.tile([128, 1024], fp32, name="st")
            pt = pt_pool.tile([128, 1024], mm_dt, name="pt")
            for idx in range(2):
                r = idx * 64
                nc.tensor.matmul(
                    st[:, idx * 512 + off:(idx + 1) * 512],
                    kt_sb[cc][r:r + 64, kt * 128:(kt + 1) * 128],
                    qt_sb[cc][r:r + 64, g4 * 512 + off:(g4 + 1) * 512],
                    start=True,
                    stop=True,
                )
            if not diag:
                nc.scalar.activation(out=pt, in_=st, func=EXP, scale=0.125)
            else:
                st3 = st.rearrange("p (i c) -> p i c", i=2)
                pt3 = pt.rearrange("p (i c) -> p i c", i=2)
                nc.scalar.activation(
                    out=pt3[:, :, off:512],
                    in_=st3[:, :, off:512],
                    func=EXP,
                    scale=0.125,
                )
                # mask only the 128-wide staircase strip (on DVE: gpsimd
                # tensor ops are slower and force ucode library reloads
                # around the partition_broadcasts)
                strip = pt3[:, :, off:off + 128]
                nc.vector.tensor_mul(
                    out=strip,
                    in0=strip,
                    in1=tri_sb[:, None, :].broadcast_to([128, 2, 128]),
                )
            return pt

        def emit_PV(g4, hp, kt, pt, ot_ps, nkt):
            diag = (kt // 4 == g4)
            off = 128 * (kt % 4) if diag else 0
            for idx in range(2):
                h = 2 * hp + idx
                if g4 == kt // 4:
                    need(("v", kt))
                nc.tensor.matmul(
                    ot_ps[idx][:, off:512],
                    v_sb[kt // 4][:, kt % 4, h, :],
                    pt[:, idx * 512 + off:(idx + 1) * 512],
                    start=(kt == 0),
                    stop=(kt == nkt - 1),
                )

        PACE = [700, 600, 350, 300]

        for g4 in range(4):
            # refill the filler queue (ordered by first-need time). C(g) is
            # queued at g4=g+1 so the 4MB y writeback spreads over the whole
            # run instead of jamming the DMA queues in the last 13us.
            if g4 == 0:
                queue += qk_units(kt_sb, xk_sb, wk_sb, 2, 1, 0)
                queue += qk_units(kt_sb, xk_sb, wk_sb, 3, 1, 1)
                queue += qk_units(qt_sb, xq_sb, wq_sb, 0, 1, 0)
                queue += qk_units(qt_sb, xq_sb, wq_sb, 1, 1, 1)
            elif g4 == 1:
                queue += v_units(4) + v_units(5) + v_units(6) + v_units(7)
                queue += qk_units(kt_sb, xk_sb, wk_sb, 2, 2, 0)
                queue += qk_units(kt_sb, xk_sb, wk_sb, 3, 2, 1)
                queue += qk_units(qt_sb, xq_sb, wq_sb, 0, 2, 0)
                queue += qk_units(qt_sb, xq_sb, wq_sb, 1, 2, 1)
                queue += c_units(0)
            elif g4 == 2:
                queue += v_units(8) + v_units(9) + v_units(10) + v_units(11)
                queue += qk_units(kt_sb, xk_sb, wk_sb, 2, 3, 0)
                queue += qk_units(kt_sb, xk_sb, wk_sb, 3, 3, 1)
                queue += qk_units(qt_sb, xq_sb, wq_sb, 0, 3, 0)
                queue += qk_units(qt_sb, xq_sb, wq_sb, 1, 3, 1)
                queue += c_units(1)
            elif g4 == 3:
                queue += v_units(12) + v_units(13) + v_units(14) + v_units(15)
                queue += c_units(2)[:4]
                reserve.extend(c_units(2, act_copy=True)[4:])

            nkt = 4 * g4 + 4
            for hp in range(2):
                cc = hp
                need(("qk", g4, cc))
                ot_ps = [
                    psOT.tile([65, 512], fp32, name="ot0"),
                    psOT.tile([65, 512], fp32, name="ot1"),
                ]
                prev = None
                for kt in range(nkt):
                    cur = emit_S(g4, cc, kt)
                    # batched pumping: fewer PSUM-bank regime switches on
                    # the PE (HAM K-state cycles on micro-idles between
                    # mm groups)
                    if kt % 2 == 1:
                        pump_ns(2 * PACE[g4])
                    if kt > 0:
                        emit_PV(g4, hp, kt - 1, prev, ot_ps, nkt)
                    prev = cur
                # last key tile: emit each head's PV then immediately its
                # division so the DVE chain starts earlier. No pumping
                # between division emissions (transitive FIFO stalls).
                kt = nkt - 1
                diag_l = (kt // 4 == g4)
                off = 128 * (kt % 4) if diag_l else 0
                if g4 == kt // 4:
                    need(("v", kt))
                for idx in range(2):
                    h = 2 * hp + idx
                    nc.tensor.matmul(
                        ot_ps[idx][:, off:512],
                        v_sb[kt // 4][:, kt % 4, h, :],
                        prev[:, idx * 512 + off:(idx + 1) * 512],
                        start=(kt == 0),
                        stop=True,
                    )
                # softmax division, latency-interleaved across engines:
                # z row copy on ACT (PSUM->SBUF; ACT is idle right here),
                # recip on the [1,512] row (DVE), THEN broadcast the
                # reciprocal (GPSIMD), mul on DVE.
                # NB: reciprocal reading PSUM directly is silently wrong.
                zrow = [None, None]
                zrc = [None, None]
                zbr = [None, None]
                for idx in range(2):
                    zrow[idx] = zr_pool.tile([1, 512], fp32, name=f"zrow{idx}")
                    nc.scalar.activation(
                        out=zrow[idx], in_=ot_ps[idx][64:65, :],
                        func=mybir.ActivationFunctionType.Copy, bias=0.0,
                    )
                    zrc[idx] = zc_pool.tile([1, 512], fp32, name=f"zrc{idx}")
                    nc.vector.reciprocal_approx_fast(out=zrc[idx], in_=zrow[idx])
                for idx in range(2):
                    zbr[idx] = zb_pool.tile([64, 512], fp32, name=f"zbr{idx}")
                    nc.gpsimd.partition_broadcast(out_ap=zbr[idx], in_ap=zrc[idx])
                if (g4, hp) != (3, 1):
                    for idx in range(2):
                        nc.vector.tensor_mul(
                            out=ot_sb[cc][g4][idx * 64:idx * 64 + 64, :],
                            in0=ot_ps[idx][0:64, :],
                            in1=zbr[idx],
                        )
                    pump_ns(3000)
                else:
                    # final divisions: muls deferred to the tail, split per
                    # 128-col q-block and interleaved with the C(3) cc1
                    # matmuls so each C group starts as soon as ITS block
                    # is divided
                    last_ot_ps, last_zbr = ot_ps, zbr

        # ---- tail: reserved C units bridge the last division window; then
        # C(3) in [128,1024] psS groups (double-buffered)
        for _, _, f in reserve:
            f()
        while queue:
            queue.pop(0)[2]()
        # C(3): the cc2=0 halves only need hp=0's divisions (done ~2.5us
        # before hp=1's), so they are the ONLY work that becomes ready
        # DURING the final division window -- emit them first so the PE has
        # something to chew on while DVE runs the last chain. lt 12/13 in
        # the two psS slots, lt 14 in the two psA slots.
        st3 = {}
        for lt in (12, 13):
            st3[lt] = psS.tile([128, 1024], fp32, name="st")
            for dg in range(2):
                nc.tensor.matmul(
                    st3[lt][:, dg * 512:(dg + 1) * 512],
                    ot_sb[0][3][:, (lt % 4) * 128:(lt % 4 + 1) * 128],
                    wo_sb[0][:, dg * 512:(dg + 1) * 512],
                    start=True, stop=False,
                )
        for dg in range(2):
            st3[(14, dg)] = psA.tile([128, 512], fp32, name="ps")
            nc.tensor.matmul(
                st3[(14, dg)],
                ot_sb[0][3][:, 2 * 128:3 * 128],
                wo_sb[0][:, dg * 512:(dg + 1) * 512],
                start=True, stop=False,
            )
        # per-block division muls interleaved with the cc1 C matmuls
        for blk in range(4):
            bs = slice(blk * 128, (blk + 1) * 128)
            for idx in range(2):
                nc.vector.tensor_mul(
                    out=ot_sb[1][3][idx * 64:idx * 64 + 64, bs],
                    in0=last_ot_ps[idx][0:64, bs],
                    in1=last_zbr[idx][:, bs],
                )
            lt = 12 + blk
            if lt == 15:
                st15 = psS.tile([128, 1024], fp32, name="st")
                for dg in range(2):
                    for cc2 in range(2):
                        nc.tensor.matmul(
                            st15[:, dg * 512:(dg + 1) * 512],
                            ot_sb[cc2][3][:, bs],
                            wo_sb[cc2][:, dg * 512:(dg + 1) * 512],
                            start=(cc2 == 0),
                            stop=(cc2 == 1),
                        )
                yt = y_pool.tile([128, 1024], mm_dt, name="yt")
                nc.vector.tensor_copy(out=yt[:, 0:512], in_=st15[:, 0:512])
                nc.scalar.activation(
                    out=yt[:, 512:1024], in_=st15[:, 512:1024],
                    func=mybir.ActivationFunctionType.Copy,
                    bias=0.0,
                )
                nc.scalar.dma_start(out=y_view[:, 15, :], in_=yt)
            elif lt == 14:
                for dg in range(2):
                    nc.tensor.matmul(
                        st3[(14, dg)],
                        ot_sb[1][3][:, bs],
                        wo_sb[1][:, dg * 512:(dg + 1) * 512],
                        start=False, stop=True,
                    )
                    yt = y_pool.tile([128, 512], mm_dt, name="yh")
                    if dg == 0:
                        nc.vector.tensor_copy(out=yt, in_=st3[(14, dg)])
                    else:
                        nc.scalar.activation(
                            out=yt, in_=st3[(14, dg)],
                            func=mybir.ActivationFunctionType.Copy, bias=0.0,
                        )
                    nc.scalar.dma_start(
                        out=y_view[:, 14, dg * 512:(dg + 1) * 512], in_=yt)
            else:
                for dg in range(2):
                    nc.tensor.matmul(
                        st3[lt][:, dg * 512:(dg + 1) * 512],
                        ot_sb[1][3][:, bs],
                        wo_sb[1][:, dg * 512:(dg + 1) * 512],
                        start=False, stop=True,
                    )
                yt = y_pool.tile([128, 1024], mm_dt, name="yt")
                nc.vector.tensor_copy(out=yt[:, 0:512], in_=st3[lt][:, 0:512])
                nc.scalar.activation(
                    out=yt[:, 512:1024], in_=st3[lt][:, 512:1024],
                    func=mybir.ActivationFunctionType.Copy,
                    bias=0.0,
                )
                nc.scalar.dma_start(out=y_view[:, lt, :], in_=yt)

    nc.compile()
    return nc


def _get_nc(mm_dt: str):
    if mm_dt not in _CACHE:
        _CACHE[mm_dt] = build_nc(mm_dt)
    return _CACHE[mm_dt]


def kernel(q, k, v, mask, Wq, bq, Wk, bk, Wv, bv, Wo, bo, _trace=False):
    nc = _get_nc(MM_DT)

    # DMA-friendly host swizzles (contiguous per partition on-device):
    #  x  [D,L] -> [128, dc*L]            A[p, dc*L+c]       = xT[dc*128+p, c]
    #  xv [D,L] -> [128, g*dc*512]        A[p,(g*8+dc)*512+c] = xT[dc*128+p, g*512+c]
    #  w  [D,CPC] -> [128, dc*CPC]        A[p, dc*CPC+c]     = W[dc*128+p, c]
    def _sw_x(xt):
        return np.ascontiguousarray(
            xt.reshape(8, 128, L).transpose(1, 0, 2).reshape(128, 8 * L))

    def _sw_xv(xt):
        return np.ascontiguousarray(
            xt.reshape(8, 128, 4, 512).transpose(1, 2, 0, 3).reshape(128, 4 * 8 * 512))

    def _sw_w(w):
        return np.ascontiguousarray(
            w.reshape(8, 128, CPC).transpose(1, 0, 2).reshape(128, 8 * CPC))

    in_maps = []
    for c in range(NCORES):
        b = c // 4
        g = c % 4
        s = slice(g * CPC, (g + 1) * CPC)
        bq_s = np.ascontiguousarray(bq[s]).reshape(2, 128).T
        bk_s = np.ascontiguousarray(bk[s]).reshape(2, 128).T
        qT = np.ascontiguousarray(q[b].T).astype(NP_MM)
        kT = np.ascontiguousarray(k[b].T).astype(NP_MM)
        vT = np.ascontiguousarray(v[b].T).astype(NP_MM)
        in_maps.append({
            "xq": _sw_x(qT),
            "xk": _sw_x(kT),
            "xv": _sw_xv(vT),
            "wq": _sw_w(np.ascontiguousarray(Wq[:, s]).astype(NP_MM)),
            "wk": _sw_w(np.ascontiguousarray(Wk[:, s]).astype(NP_MM)),
            "wv": _sw_w(np.ascontiguousarray(Wv[:, s]).astype(NP_MM)),
            "wo": np.ascontiguousarray(Wo[s, :]).astype(NP_MM),
            "bqk": np.ascontiguousarray(
                np.concatenate([bq_s, bk_s], axis=1)).astype(np.float32),
        })

    res = run_bass_kernel_spmd(nc, in_maps, list(range(NCORES)), trace=_trace)

    # host gather: out[b] = sum_g y_core(b,g) + (bo + bv @ Wo)
    const = (bo + bv.astype(np.float64) @ Wo.astype(np.float64)).astype(np.float64)
    out = np.zeros((B, L, D), np.float64)
    for c in range(NCORES):
        out[c // 4] += res.results[c]["y"].astype(np.float64)
    out += const[None, None, :]
    kernel.last_exec_time_ns = res.exec_time_ns
    return out.astype(np.float32)
